# revision 55
# baseline (speedup 1.0000x reference)
"""Block-wise (128x128) min/max quantization observer kernel for TRN2.

Computes per-block scale / zero_point over an [8192, 8192] f32 tensor and
replicates each block's params over its 128x128 region, returning full-shape
scale (f32) and zero_point (i32) tensors — matching the jax reference
bit-exactly on the non-degenerate path.

Sharding: 8 NeuronCores, each handles a 1024-row stripe (8 row-blocks),
fully independent (embarrassingly parallel over row-blocks).

The device kernel streams the stripe once (32 MiB/core of HBM reads, the
~94us/core HBM roofline) and emits only the per-block params; the
block->full-tensor replication (a pure broadcast) happens on the host during
unsharding.

Design (v3): DVE is the only engine whose free-dim min/max reduce this
toolchain compiles (gpsimd TensorTensor/InstPool fail walrus's Pool-opcode
check; TensorTensor on Activation fails the BIR verifier; the fused
tensor_tensor_reduce faults on HW), so the two streaming passes
(~121us/core, 1 elem/cycle/lane) ARE the critical path — the 14 input DMAs
finish with ~25us of slack. Everything else is shaped to keep DVE busy
end-to-end: row-block 0 streams in 8/8/16/16/16-block chunks so DVE's first
reduce starts ~4us in, row-block 1 in halves to bridge the DMA ramp, the
rest as whole 4 MiB contiguous tiles to amortize the per-instruction SBUF
bubble. Cross-partition reduction goes through PE transposes into PSUM
(DVE cannot read mismatched base partitions) with one free-dim reduce per
128-block group — groups 0-2 run mid-stream, only group 3 sits on the
tail. The qparam math runs on the resulting [128, 4] layout (block =
partition), so the whole chain including the bit-exact iterative
reciprocal costs ~1us, and one final PE transpose puts scale+zp straight
into flat DRAM block order.
"""

import numpy as np

ROWS, COLS = 8192, 8192
BR, BC = 128, 128
N_CORES = 8
ROWS_PER_CORE = ROWS // N_CORES          # 1024
RB_PER_CORE = ROWS_PER_CORE // BR        # 8 row-blocks per core
NCB = COLS // BC                         # 64 col-blocks
NB_TOT = RB_PER_CORE * NCB               # 512 blocks per core

# 1.5 * 2**23: adding/subtracting this in fp32 rounds to nearest-even integer
# for |x| < 2**22 (the round-half-even jnp.round behavior).
RNE_MAGIC = 12582912.0
RNE_MAGIC_M128 = RNE_MAGIC - 128.0       # exact in fp32
# fp32(1/255), the multiplier neuron's fp32 divide-by-255 uses.
R255 = float(np.float32(1.0) / np.float32(255.0))

_CACHE = {}


def _build(reps=1, pin_bufs=None, probe=None):
    # probe: None = full kernel; "nodve" = DMA only; "nodma" = memset input
    #   tiles instead of DMA (compute-only timing).
    import concourse.bacc as bacc
    import concourse.tile as tile
    import concourse.mybir as mybir

    f32 = mybir.dt.float32
    i32 = mybir.dt.int32
    Alu = mybir.AluOpType
    X = mybir.AxisListType.X

    nc = bacc.Bacc(
        "TRN2",
        target_bir_lowering=False,
        debug=False,
        num_devices=N_CORES,
    )
    obs = nc.dram_tensor(
        "observed", [ROWS_PER_CORE, COLS], f32, kind="ExternalInput"
    ).ap()
    # Output layout: 512 per-block values in block order b = rb*64 + cb,
    # viewed as [16, 32] (partition g = b // 32, free j = b % 32).
    scale_dram = nc.dram_tensor(
        "scale_b", [RB_PER_CORE, NCB], f32, kind="ExternalOutput"
    ).ap()
    zp_dram = nc.dram_tensor(
        "zp_b", [RB_PER_CORE, NCB], i32, kind="ExternalOutput"
    ).ap()

    # (rb, col_block_start, n_col_blocks) chunks. DVE is the critical
    # resource (the DMA finishes with ~45us of slack), so chunk sizes are
    # chosen to keep DVE busy end-to-end: row-block 0 streams as four
    # [128, 2048] quarters so DVE's first reduce starts ~5us in (a full
    # tile's DMA would make it ~14us), row-block 1 as two halves to bridge
    # the DMA ramp without a bubble, and the rest as full [128, 8192] tiles
    # to amortize the ~58-cycle per-instruction SBUF bubble.
    chunks = []
    for rb in range(RB_PER_CORE):
        if rb == 0:
            # 4+4+8+16+16+16 blocks: DVE's first reduce starts ~3us in.
            chunks += [(rb, 0, 4), (rb, 4, 4), (rb, 8, 8), (rb, 16, 16),
                       (rb, 32, 16), (rb, 48, 16)]
        elif rb == 1:
            chunks.append((rb, 0, 32))
            chunks.append((rb, 32, 32))
        else:
            chunks.append((rb, 0, NCB))
    if pin_bufs is None:
        pin_bufs = 4

    from concourse import masks

    with tile.TileContext(nc) as tc:
        with (
            tc.tile_pool(name="pin", bufs=pin_bufs) as pin,
            tc.tile_pool(name="ppart", bufs=2) as ppart,
            tc.tile_pool(name="pid", bufs=1) as pid,
            tc.tile_pool(name="ppsum", bufs=2,
                         space=tile.bass.MemorySpace.PSUM) as ppsum,
            tc.tile_pool(name="psmall", bufs=1) as psmall,
        ):
            identity = pid.tile([BR, BR], f32, name="ident")
            masks.make_identity(nc, identity)

            # Chunk index at which row-block r's last chunk is processed.
            rb_last_ci = {}
            for ci, (rb, cb0, ncb) in enumerate(chunks):
                if cb0 + ncb == NCB:
                    rb_last_ci[rb] = ci

            for _rep in range(reps):
                # Streaming partials; free index = rb*64 + cb.
                # pnmin holds -min so every later reduce is a max.
                pmax = ppart.tile([BR, NB_TOT], f32, name="pmax")
                pnmin = ppart.tile([BR, NB_TOT], f32, name="pnmin")

                # Cross-partition reduction via PE transpose: blocks
                # [128g, 128(g+1)) of src transpose through PSUM (partition
                # dim becomes the free dim), then one free-dim reduce leaves
                # res[c, g] = reduced block b = 128g + c. DVE cannot read
                # mismatched base partitions, so PE does the lane move.
                res_mx = psmall.tile([BR, 4], f32, name="resmx")
                res_mn = psmall.tile([BR, 4], f32, name="resmn")

                def part_reduce_group(g):
                    for nm, src, res in (
                        ("x", pmax, res_mx), ("n", pnmin, res_mn)
                    ):
                        pst = ppsum.tile([BR, BR], f32, name=f"pst{nm}")
                        nc.tensor.transpose(
                            pst, src[:, g * BR:(g + 1) * BR], identity
                        )
                        nc.vector.tensor_reduce(
                            out=res[:, g:g + 1], in_=pst, axis=X, op=Alu.max,
                        )

                for ci, (rb, cb0, ncb) in enumerate(chunks):
                    r0 = rb * BR
                    c0 = cb0 * BC
                    cw = ncb * BC
                    x = pin.tile([BR, cw], f32, name="x")
                    if probe == "nodma":
                        nc.scalar.memzero(x)
                    else:
                        eng = nc.sync if ci % 2 == 0 else nc.scalar
                        eng.dma_start(out=x, in_=obs[r0:r0 + BR, c0:c0 + cw])
                    if probe == "nodve":
                        continue
                    x3 = x.rearrange("p (b c) -> p b c", c=BC)
                    b0 = rb * NCB + cb0
                    # Streaming reduces all run on DVE: it is the only engine
                    # whose free-dim reduce this toolchain compiles (Pool
                    # TensorTensor/InstPool fail the walrus Pool-opcode
                    # check; TensorTensor on Activation fails the verifier).
                    nc.vector.tensor_reduce(
                        out=pmax[:, b0:b0 + ncb], in_=x3, axis=X,
                        op=Alu.max,
                    )
                    nc.vector.tensor_reduce(
                        out=pnmin[:, b0:b0 + ncb], in_=x3,
                        axis=X, op=Alu.min, negate=True,
                    )
                    # Blocks [128g, 128(g+1)) are fully reduced once row-
                    # block 2g+1's chunks are done; run those groups'
                    # PE-transpose reductions mid-stream so only group 3
                    # remains on the critical tail.
                    for g in range(3):
                        if ci == rb_last_ci[2 * g + 1]:
                            part_reduce_group(g)

                if probe == "nodve":
                    # Ship garbage out so outputs exist (timing probe only).
                    sb = psmall.tile([4, BR], f32, name="sbp")
                    nc.vector.memset(sb, 0.0)
                    zb = psmall.tile([4, BR], i32, name="zbp")
                    nc.vector.tensor_copy(out=zb, in_=sb)
                    nc.sync.dma_start(
                        out=scale_dram.rearrange("a b -> (a b)")
                        .rearrange("(p f) -> p f", f=BR), in_=sb,
                    )
                    nc.scalar.dma_start(
                        out=zp_dram.rearrange("a b -> (a b)")
                        .rearrange("(p f) -> p f", f=BR), in_=zb,
                    )
                    continue

                # Finish the cross-partition reduce: blocks [384, 512).
                part_reduce_group(3)
                bmax_r, nbmin_r = res_mx, res_mn
                npart, nfree = BR, 4

                # Per-block qparams (identical math to the jax reference) on
                # the [128, 4] layout: every op has free size 4, so the whole
                # chain (including the bit-exact iterative reciprocal) costs
                # well under 1us.
                rng = psmall.tile([npart, nfree], f32, name="rng")
                nc.vector.tensor_tensor(rng, bmax_r, nbmin_r, Alu.add)
                deg = psmall.tile([npart, nfree], f32, name="deg")
                nc.vector.tensor_scalar(deg, rng, 0.0, None, Alu.is_equal)
                # Packed [128, 8]: scale in cols 0:4, zp (fp32) in cols 4:8,
                # so one PE transpose yields the DRAM block order for both.
                packed = psmall.tile([BR, 8], f32, name="packed")
                scale_b = packed[:, 0:4]
                # scale = rng * (1/255) + deg  (== 1.0 when degenerate).
                nc.vector.scalar_tensor_tensor(
                    scale_b, rng, R255, deg, Alu.mult, Alu.add
                )
                # t2 = -bmin/scale = nbmin * exact_reciprocal(scale)
                rcp = psmall.tile([npart, nfree], f32, name="rcp")
                nc.vector.reciprocal(rcp, scale_b)
                t2 = psmall.tile([npart, nfree], f32, name="t2")
                nc.vector.tensor_tensor(t2, nbmin_r, rcp, Alu.mult)
                # zpr = round_half_even(t2 - 128) = qmin - bmin/scale rounded
                zpr = psmall.tile([npart, nfree], f32, name="zpr")
                nc.vector.tensor_scalar(
                    zpr, t2, RNE_MAGIC_M128, RNE_MAGIC, Alu.add, Alu.subtract
                )
                notdeg = psmall.tile([npart, nfree], f32, name="notdeg")
                nc.vector.tensor_scalar(
                    notdeg, deg, -1.0, 1.0, Alu.mult, Alu.add
                )
                # zp = zpr * (1 - deg) (integer-valued fp32)
                nc.vector.tensor_tensor(packed[:, 4:8], zpr, notdeg, Alu.mult)

                # [128, 8] -> [8, 128]: row j < 4 holds scale for blocks
                # 128j..128j+127, rows 4:8 the same for zp — exactly the
                # flat DRAM block order.
                pres = ppsum.tile([8, BR], f32, name="pres")
                nc.tensor.transpose(pres, packed, identity)
                scale_sb = psmall.tile([4, BR], f32, name="scale_sb")
                nc.vector.tensor_copy(out=scale_sb, in_=pres[0:4, :])
                # PSUM reads must start at partition 0: convert all 8 rows
                # (0:4 are scale-as-int garbage, never read) and DMA 4:8.
                zp8 = psmall.tile([8, BR], i32, name="zp8")
                nc.vector.tensor_copy(out=zp8, in_=pres)
                nc.sync.dma_start(
                    out=scale_dram.rearrange("a b -> (a b)")
                    .rearrange("(p f) -> p f", f=BR),
                    in_=scale_sb,
                )
                nc.scalar.dma_start(
                    out=zp_dram.rearrange("a b -> (a b)")
                    .rearrange("(p f) -> p f", f=BR),
                    in_=zp8[4:8, :],
                )

    nc.compile()
    return nc


def _get_nc():
    if "nc" not in _CACHE:
        _CACHE["nc"] = _build()
    return _CACHE["nc"]


def _make_runner(nc=None):
    """Jitted shard_map callable: full [8192,8192] in -> per-block outs.

    Binds the bass_exec primitive directly, sharding axis 0 across the
    8 cores.
    """
    import jax
    import numpy as _np
    from jax.sharding import Mesh, PartitionSpec
    from jax.experimental.shard_map import shard_map
    from concourse import bass2jax
    import concourse.mybir as mybir

    if nc is None:
        nc = _get_nc()
    bass2jax.install_neuronx_cc_hook()

    partition_name = (
        nc.partition_id_tensor.name if nc.partition_id_tensor else None
    )
    in_names, out_names, out_avals = [], [], []
    for alloc in nc.m.functions[0].allocations:
        if not isinstance(alloc, mybir.MemoryLocationSet):
            continue
        name = alloc.memorylocations[0].name
        if alloc.kind == "ExternalInput":
            if name != partition_name:
                in_names.append(name)
        elif alloc.kind == "ExternalOutput":
            out_names.append(name)
            out_avals.append(
                jax.core.ShapedArray(
                    tuple(alloc.tensor_shape), mybir.dt.np(alloc.dtype)
                )
            )
    bind_in_names = list(in_names)
    if partition_name is not None:
        bind_in_names.append(partition_name)

    def _body(*args):
        operands = list(args)
        if partition_name is not None:
            operands.append(bass2jax.partition_id_tensor())
        outs = bass2jax._bass_exec_p.bind(
            *operands,
            out_avals=tuple(out_avals),
            in_names=tuple(bind_in_names),
            out_names=tuple(out_names),
            lowering_input_output_aliases=(),
            sim_require_finite=True,
            sim_require_nnan=True,
            nc=nc,
        )
        return tuple(outs)

    devices = jax.devices()[:N_CORES]
    assert len(devices) == N_CORES
    mesh = Mesh(_np.asarray(devices), ("core",))
    fn = jax.jit(
        shard_map(
            _body,
            mesh=mesh,
            in_specs=(PartitionSpec("core"),) * len(in_names),
            out_specs=(PartitionSpec("core"),) * len(out_names),
            check_rep=False,
        )
    )
    return fn, out_names, mesh


def _get_runner():
    if "runner" not in _CACHE:
        _CACHE["runner"] = _make_runner()
    return _CACHE["runner"]


def _expand(scale_blocks, zp_blocks):
    """[64, 64] per-block params -> full [8192, 8192] outputs."""
    nrb, ncb = ROWS // BR, COLS // BC
    scale = np.broadcast_to(
        scale_blocks.reshape(nrb, 1, ncb, 1), (nrb, BR, ncb, BC)
    ).reshape(ROWS, COLS)
    zp = np.broadcast_to(
        zp_blocks.reshape(nrb, 1, ncb, 1), (nrb, BR, ncb, BC)
    ).reshape(ROWS, COLS)
    return np.ascontiguousarray(scale), np.ascontiguousarray(zp)


def _run_fallback(observed):
    """Slower but battle-tested path via run_bass_kernel_spmd."""
    from concourse.bass_utils import run_bass_kernel_spmd

    nc = _get_nc()
    in_maps = [
        {
            "observed": np.ascontiguousarray(
                observed[i * ROWS_PER_CORE : (i + 1) * ROWS_PER_CORE]
            )
        }
        for i in range(N_CORES)
    ]
    res = run_bass_kernel_spmd(nc, in_maps, list(range(N_CORES)))
    scale_blocks = np.concatenate(
        [res.results[i]["scale_b"] for i in range(N_CORES)], axis=0
    )
    zp_blocks = np.concatenate(
        [res.results[i]["zp_b"] for i in range(N_CORES)], axis=0
    )
    return _expand(scale_blocks, zp_blocks)


def kernel(**inputs):
    observed = np.asarray(inputs["observed"], dtype=np.float32)
    assert observed.shape == (ROWS, COLS)
    try:
        fn, out_names, _ = _get_runner()
        outs = fn(observed)
        by_name = dict(zip(out_names, outs))
        scale_blocks = np.asarray(by_name["scale_b"])
        zp_blocks = np.asarray(by_name["zp_b"])
    except Exception:
        return _run_fallback(observed)
    return _expand(scale_blocks, zp_blocks)


# revision 56
# speedup vs baseline: 1.2534x; 1.2534x over previous
"""Block-wise (128x128) min/max quantization observer kernel for TRN2.

Computes per-block scale / zero_point over an [8192, 8192] f32 tensor and
replicates each block's params over its 128x128 region, returning full-shape
scale (f32) and zero_point (i32) tensors — matching the jax reference
bit-exactly on the non-degenerate path.

Sharding: 8 NeuronCores, each handles a 1024-row stripe (8 row-blocks),
fully independent (embarrassingly parallel over row-blocks).

The device kernel streams the stripe once (32 MiB/core of HBM reads, the
~94us/core HBM roofline) and emits only the per-block params; the
block->full-tensor replication (a pure broadcast) happens on the host during
unsharding.

Design (v3): DVE is the only engine whose free-dim min/max reduce this
toolchain compiles (gpsimd TensorTensor/InstPool fail walrus's Pool-opcode
check; TensorTensor on Activation fails the BIR verifier; the fused
tensor_tensor_reduce faults on HW), so the two streaming passes
(~121us/core, 1 elem/cycle/lane) ARE the critical path — the 14 input DMAs
finish with ~25us of slack. Everything else is shaped to keep DVE busy
end-to-end: row-block 0 streams in 8/8/16/16/16-block chunks so DVE's first
reduce starts ~4us in, row-block 1 in halves to bridge the DMA ramp, the
rest as whole 4 MiB contiguous tiles to amortize the per-instruction SBUF
bubble. Cross-partition reduction goes through PE transposes into PSUM
(DVE cannot read mismatched base partitions) with one free-dim reduce per
128-block group — groups 0-2 run mid-stream, only group 3 sits on the
tail. The qparam math runs on the resulting [128, 4] layout (block =
partition), so the whole chain including the bit-exact iterative
reciprocal costs ~1us, and one final PE transpose puts scale+zp straight
into flat DRAM block order.
"""

import numpy as np

ROWS, COLS = 8192, 8192
BR, BC = 128, 128
N_CORES = 8
ROWS_PER_CORE = ROWS // N_CORES          # 1024
RB_PER_CORE = ROWS_PER_CORE // BR        # 8 row-blocks per core
NCB = COLS // BC                         # 64 col-blocks
NB_TOT = RB_PER_CORE * NCB               # 512 blocks per core

# 1.5 * 2**23: adding/subtracting this in fp32 rounds to nearest-even integer
# for |x| < 2**22 (the round-half-even jnp.round behavior).
RNE_MAGIC = 12582912.0
RNE_MAGIC_M128 = RNE_MAGIC - 128.0       # exact in fp32
# fp32(1/255), the multiplier neuron's fp32 divide-by-255 uses.
R255 = float(np.float32(1.0) / np.float32(255.0))

_CACHE = {}


def _build(reps=1, pin_bufs=None, probe=None):
    # probe: None = full kernel; "nodve" = DMA only; "nodma" = memset input
    #   tiles instead of DMA (compute-only timing).
    import concourse.bacc as bacc
    import concourse.tile as tile
    import concourse.mybir as mybir

    f32 = mybir.dt.float32
    i32 = mybir.dt.int32
    Alu = mybir.AluOpType
    X = mybir.AxisListType.X

    nc = bacc.Bacc(
        "TRN2",
        target_bir_lowering=False,
        debug=False,
        num_devices=N_CORES,
    )
    obs = nc.dram_tensor(
        "observed", [ROWS_PER_CORE, COLS], f32, kind="ExternalInput"
    ).ap()
    # Output layout: 512 per-block values in block order b = rb*64 + cb,
    # viewed as [16, 32] (partition g = b // 32, free j = b % 32).
    scale_dram = nc.dram_tensor(
        "scale_b", [RB_PER_CORE, NCB], f32, kind="ExternalOutput"
    ).ap()
    zp_dram = nc.dram_tensor(
        "zp_b", [RB_PER_CORE, NCB], i32, kind="ExternalOutput"
    ).ap()

    # (rb, col_block_start, n_col_blocks) chunks. DVE is the critical
    # resource (the DMA finishes with ~45us of slack), so chunk sizes are
    # chosen to keep DVE busy end-to-end: row-block 0 streams as four
    # [128, 2048] quarters so DVE's first reduce starts ~5us in (a full
    # tile's DMA would make it ~14us), row-block 1 as two halves to bridge
    # the DMA ramp without a bubble, and the rest as full [128, 8192] tiles
    # to amortize the ~58-cycle per-instruction SBUF bubble.
    chunks = []
    for rb in range(RB_PER_CORE):
        if rb == 0:
            # 8+8+16+16+16 blocks: DVE's first reduce starts ~4us in.
            chunks += [(rb, 0, 8), (rb, 8, 8), (rb, 16, 16), (rb, 32, 16),
                       (rb, 48, 16)]
        elif rb == 1:
            chunks.append((rb, 0, 32))
            chunks.append((rb, 32, 32))
        else:
            chunks.append((rb, 0, NCB))
    if pin_bufs is None:
        pin_bufs = 4

    from concourse import masks

    with tile.TileContext(nc) as tc:
        with (
            tc.tile_pool(name="pin", bufs=pin_bufs) as pin,
            tc.tile_pool(name="ppart", bufs=2) as ppart,
            tc.tile_pool(name="pid", bufs=1) as pid,
            tc.tile_pool(name="ppsum", bufs=2,
                         space=tile.bass.MemorySpace.PSUM) as ppsum,
            tc.tile_pool(name="psmall", bufs=1) as psmall,
        ):
            identity = pid.tile([BR, BR], f32, name="ident")
            masks.make_identity(nc, identity)

            # Chunk index at which row-block r's last chunk is processed.
            rb_last_ci = {}
            for ci, (rb, cb0, ncb) in enumerate(chunks):
                if cb0 + ncb == NCB:
                    rb_last_ci[rb] = ci

            for _rep in range(reps):
                # Streaming partials; free index = rb*64 + cb.
                # pnmin holds -min so every later reduce is a max.
                pmax = ppart.tile([BR, NB_TOT], f32, name="pmax")
                pnmin = ppart.tile([BR, NB_TOT], f32, name="pnmin")

                # Cross-partition reduction via PE transpose: blocks
                # [128g, 128(g+1)) of src transpose through PSUM (partition
                # dim becomes the free dim), then one free-dim reduce leaves
                # res[c, g] = reduced block b = 128g + c. DVE cannot read
                # mismatched base partitions, so PE does the lane move.
                res_mx = psmall.tile([BR, 4], f32, name="resmx")
                res_mn = psmall.tile([BR, 4], f32, name="resmn")

                def part_reduce_group(g):
                    for nm, src, res in (
                        ("x", pmax, res_mx), ("n", pnmin, res_mn)
                    ):
                        pst = ppsum.tile([BR, BR], f32, name=f"pst{nm}")
                        nc.tensor.transpose(
                            pst, src[:, g * BR:(g + 1) * BR], identity
                        )
                        nc.vector.tensor_reduce(
                            out=res[:, g:g + 1], in_=pst, axis=X, op=Alu.max,
                        )

                for ci, (rb, cb0, ncb) in enumerate(chunks):
                    r0 = rb * BR
                    c0 = cb0 * BC
                    cw = ncb * BC
                    x = pin.tile([BR, cw], f32, name="x")
                    if probe == "nodma":
                        nc.scalar.memzero(x)
                    else:
                        eng = nc.sync if ci % 2 == 0 else nc.scalar
                        eng.dma_start(out=x, in_=obs[r0:r0 + BR, c0:c0 + cw])
                    if probe == "nodve":
                        continue
                    x3 = x.rearrange("p (b c) -> p b c", c=BC)
                    b0 = rb * NCB + cb0
                    # Streaming reduces all run on DVE: it is the only engine
                    # whose free-dim reduce this toolchain compiles (Pool
                    # TensorTensor/InstPool fail the walrus Pool-opcode
                    # check; TensorTensor on Activation fails the verifier).
                    nc.vector.tensor_reduce(
                        out=pmax[:, b0:b0 + ncb], in_=x3, axis=X,
                        op=Alu.max,
                    )
                    nc.vector.tensor_reduce(
                        out=pnmin[:, b0:b0 + ncb], in_=x3,
                        axis=X, op=Alu.min, negate=True,
                    )
                    # Blocks [128g, 128(g+1)) are fully reduced once row-
                    # block 2g+1's chunks are done; run those groups'
                    # PE-transpose reductions mid-stream so only group 3
                    # remains on the critical tail.
                    for g in range(3):
                        if ci == rb_last_ci[2 * g + 1]:
                            part_reduce_group(g)

                if probe == "nodve":
                    # Ship garbage out so outputs exist (timing probe only).
                    sb = psmall.tile([4, BR], f32, name="sbp")
                    nc.vector.memset(sb, 0.0)
                    zb = psmall.tile([4, BR], i32, name="zbp")
                    nc.vector.tensor_copy(out=zb, in_=sb)
                    nc.sync.dma_start(
                        out=scale_dram.rearrange("a b -> (a b)")
                        .rearrange("(p f) -> p f", f=BR), in_=sb,
                    )
                    nc.scalar.dma_start(
                        out=zp_dram.rearrange("a b -> (a b)")
                        .rearrange("(p f) -> p f", f=BR), in_=zb,
                    )
                    continue

                # Finish the cross-partition reduce: blocks [384, 512).
                part_reduce_group(3)
                bmax_r, nbmin_r = res_mx, res_mn
                npart, nfree = BR, 4

                # Per-block qparams (identical math to the jax reference) on
                # the [128, 4] layout: every op has free size 4, so the whole
                # chain (including the bit-exact iterative reciprocal) costs
                # well under 1us.
                rng = psmall.tile([npart, nfree], f32, name="rng")
                nc.vector.tensor_tensor(rng, bmax_r, nbmin_r, Alu.add)
                deg = psmall.tile([npart, nfree], f32, name="deg")
                nc.vector.tensor_scalar(deg, rng, 0.0, None, Alu.is_equal)
                # Packed [128, 8]: scale in cols 0:4, zp (fp32) in cols 4:8,
                # so one PE transpose yields the DRAM block order for both.
                packed = psmall.tile([BR, 8], f32, name="packed")
                scale_b = packed[:, 0:4]
                # scale = rng * (1/255) + deg  (== 1.0 when degenerate).
                nc.vector.scalar_tensor_tensor(
                    scale_b, rng, R255, deg, Alu.mult, Alu.add
                )
                # t2 = -bmin/scale = nbmin * exact_reciprocal(scale)
                rcp = psmall.tile([npart, nfree], f32, name="rcp")
                nc.vector.reciprocal(rcp, scale_b)
                t2 = psmall.tile([npart, nfree], f32, name="t2")
                nc.vector.tensor_tensor(t2, nbmin_r, rcp, Alu.mult)
                # zpr = round_half_even(t2 - 128) = qmin - bmin/scale rounded
                zpr = psmall.tile([npart, nfree], f32, name="zpr")
                nc.vector.tensor_scalar(
                    zpr, t2, RNE_MAGIC_M128, RNE_MAGIC, Alu.add, Alu.subtract
                )
                notdeg = psmall.tile([npart, nfree], f32, name="notdeg")
                nc.vector.tensor_scalar(
                    notdeg, deg, -1.0, 1.0, Alu.mult, Alu.add
                )
                # zp = zpr * (1 - deg) (integer-valued fp32)
                nc.vector.tensor_tensor(packed[:, 4:8], zpr, notdeg, Alu.mult)

                # [128, 8] -> [8, 128]: row j < 4 holds scale for blocks
                # 128j..128j+127, rows 4:8 the same for zp — exactly the
                # flat DRAM block order.
                pres = ppsum.tile([8, BR], f32, name="pres")
                nc.tensor.transpose(pres, packed, identity)
                scale_sb = psmall.tile([4, BR], f32, name="scale_sb")
                nc.vector.tensor_copy(out=scale_sb, in_=pres[0:4, :])
                # PSUM reads must start at partition 0: convert all 8 rows
                # (0:4 are scale-as-int garbage, never read) and DMA 4:8.
                zp8 = psmall.tile([8, BR], i32, name="zp8")
                nc.vector.tensor_copy(out=zp8, in_=pres)
                nc.sync.dma_start(
                    out=scale_dram.rearrange("a b -> (a b)")
                    .rearrange("(p f) -> p f", f=BR),
                    in_=scale_sb,
                )
                nc.scalar.dma_start(
                    out=zp_dram.rearrange("a b -> (a b)")
                    .rearrange("(p f) -> p f", f=BR),
                    in_=zp8[4:8, :],
                )

    nc.compile()
    return nc


def _get_nc():
    if "nc" not in _CACHE:
        _CACHE["nc"] = _build()
    return _CACHE["nc"]


def _make_runner(nc=None):
    """Jitted shard_map callable: full [8192,8192] in -> per-block outs.

    Binds the bass_exec primitive directly, sharding axis 0 across the
    8 cores.
    """
    import jax
    import numpy as _np
    from jax.sharding import Mesh, PartitionSpec
    from jax.experimental.shard_map import shard_map
    from concourse import bass2jax
    import concourse.mybir as mybir

    if nc is None:
        nc = _get_nc()
    bass2jax.install_neuronx_cc_hook()

    partition_name = (
        nc.partition_id_tensor.name if nc.partition_id_tensor else None
    )
    in_names, out_names, out_avals = [], [], []
    for alloc in nc.m.functions[0].allocations:
        if not isinstance(alloc, mybir.MemoryLocationSet):
            continue
        name = alloc.memorylocations[0].name
        if alloc.kind == "ExternalInput":
            if name != partition_name:
                in_names.append(name)
        elif alloc.kind == "ExternalOutput":
            out_names.append(name)
            out_avals.append(
                jax.core.ShapedArray(
                    tuple(alloc.tensor_shape), mybir.dt.np(alloc.dtype)
                )
            )
    bind_in_names = list(in_names)
    if partition_name is not None:
        bind_in_names.append(partition_name)

    def _body(*args):
        operands = list(args)
        if partition_name is not None:
            operands.append(bass2jax.partition_id_tensor())
        outs = bass2jax._bass_exec_p.bind(
            *operands,
            out_avals=tuple(out_avals),
            in_names=tuple(bind_in_names),
            out_names=tuple(out_names),
            lowering_input_output_aliases=(),
            sim_require_finite=True,
            sim_require_nnan=True,
            nc=nc,
        )
        return tuple(outs)

    devices = jax.devices()[:N_CORES]
    assert len(devices) == N_CORES
    mesh = Mesh(_np.asarray(devices), ("core",))
    fn = jax.jit(
        shard_map(
            _body,
            mesh=mesh,
            in_specs=(PartitionSpec("core"),) * len(in_names),
            out_specs=(PartitionSpec("core"),) * len(out_names),
            check_rep=False,
        )
    )
    return fn, out_names, mesh


def _get_runner():
    if "runner" not in _CACHE:
        _CACHE["runner"] = _make_runner()
    return _CACHE["runner"]


def _expand(scale_blocks, zp_blocks):
    """[64, 64] per-block params -> full [8192, 8192] outputs."""
    nrb, ncb = ROWS // BR, COLS // BC
    scale = np.broadcast_to(
        scale_blocks.reshape(nrb, 1, ncb, 1), (nrb, BR, ncb, BC)
    ).reshape(ROWS, COLS)
    zp = np.broadcast_to(
        zp_blocks.reshape(nrb, 1, ncb, 1), (nrb, BR, ncb, BC)
    ).reshape(ROWS, COLS)
    return np.ascontiguousarray(scale), np.ascontiguousarray(zp)


def _run_fallback(observed):
    """Slower but battle-tested path via run_bass_kernel_spmd."""
    from concourse.bass_utils import run_bass_kernel_spmd

    nc = _get_nc()
    in_maps = [
        {
            "observed": np.ascontiguousarray(
                observed[i * ROWS_PER_CORE : (i + 1) * ROWS_PER_CORE]
            )
        }
        for i in range(N_CORES)
    ]
    res = run_bass_kernel_spmd(nc, in_maps, list(range(N_CORES)))
    scale_blocks = np.concatenate(
        [res.results[i]["scale_b"] for i in range(N_CORES)], axis=0
    )
    zp_blocks = np.concatenate(
        [res.results[i]["zp_b"] for i in range(N_CORES)], axis=0
    )
    return _expand(scale_blocks, zp_blocks)


def kernel(**inputs):
    observed = np.asarray(inputs["observed"], dtype=np.float32)
    assert observed.shape == (ROWS, COLS)
    try:
        fn, out_names, _ = _get_runner()
        outs = fn(observed)
        by_name = dict(zip(out_names, outs))
        scale_blocks = np.asarray(by_name["scale_b"])
        zp_blocks = np.asarray(by_name["zp_b"])
    except Exception:
        return _run_fallback(observed)
    return _expand(scale_blocks, zp_blocks)
